# revision 34
# baseline (speedup 1.0000x reference)
"""Trainium2 8-core tensor-parallel attention kernel (Bass/Tile).

Strategy (TP over heads, per the ColumnParallel/RowParallel intent):
  - Each of the 8 cores owns 1 KV head and its 4 GQA query heads.
  - Phase A1: Q/K/V projections for all 8 512-token slots (bf16 matmuls,
    fp32 PSUM), RoPE on-chip; Q^T/K^T/V kept SBUF-resident.
  - Phase A2: causal attention, head-outer order, no max-subtraction
    (scores are bounded so exp is safe in fp32); O^T = V^T P^T accumulated
    in PSUM over key blocks; per-token softmax denominators via an
    all-ones stationary matmul (replicated across partitions). The key-
    block loop is software-pipelined one deep so the ACT-engine exp never
    stalls the TensorEngine. After each head finishes, a per-head
    AllToAll fires, overlapping later heads' attention.
  - Phase C: the output projection is sequence-parallel: the AllToAlls
    convert head-sharding to token-sharding, then each core multiplies
    its 512-token slice by the FULL wo. No all-reduce; the host
    concatenates the 8 disjoint token slices.

Layout choices:
  - Activations stay transposed (X^T/Q^T/K^T/O^T: features on the 128
    partitions, tokens on the free axis) so every matmul streams 512 wide
    and fp32 DMA-transposes are never needed; V is transposed to natural
    token-major via TensorE transpose (cheap).
  - RoPE pairs are de-interleaved on the host by permuting wq/wk columns
    (even lanes then odd lanes within each head) -> RoPE is 6 vector ops
    on partition halves. The permutation cancels in Q.K^T.
  - All matmul operands are bf16 (fp32 PSUM accumulation).
"""

import math

import numpy as np

B, S, D = 2, 2048, 4096
NH, NKV, HD = 32, 8, 128
N_REP = NH // NKV
N_CORES = 8
TOK = B * S            # 4096 flattened tokens
TQ = 512               # query-block width (matmul moving free dim)
TKB = 128              # key-block width (stationary free dim)
NKC = D // 128         # 32 contraction chunks of 128
NQB = S // TQ          # 4 query blocks per batch
NSLOT = B * NQB        # 8 512-token slots
NH_LOC = NH // N_CORES  # 4 query heads per core
SCALE = 1.0 / math.sqrt(HD)

PROFILE = False
TRACE_DIR = None
LAST_EXEC_NS = None
LAST_TRACE_DIR = None

_NC_CACHE = {}


def _build_nc(mode: str):
    """mode: 'causal' (skip fully-masked blocks, triangular diag masks),
    'dense' (no mask at all), 'masked' (generic additive mask from DRAM)."""
    import concourse.tile as tile
    from concourse import bacc, mybir
    from concourse.masks import make_identity

    f32 = mybir.dt.float32
    bf16 = mybir.dt.bfloat16

    nc = bacc.Bacc(None, target_bir_lowering=False, num_devices=N_CORES)

    xT = nc.declare_dram_parameter("xT", [NSLOT, 128, NKC, TQ], bf16, isOutput=False)
    wq = nc.declare_dram_parameter("wq", [128, NKC, NH_LOC * HD], bf16, isOutput=False)
    wk = nc.declare_dram_parameter("wk", [128, NKC, HD], bf16, isOutput=False)
    wv = nc.declare_dram_parameter("wv", [128, NKC, HD], bf16, isOutput=False)
    wo = nc.declare_dram_parameter("wo", [D // TQ, 128, NH, TQ], bf16, isOutput=False)
    cosT = nc.declare_dram_parameter("cosT", [64, TOK], f32, isOutput=False)
    sinT = nc.declare_dram_parameter("sinT", [64, TOK], f32, isOutput=False)
    if mode == "causal":
        mask4 = nc.declare_dram_parameter("mask4", [128, 4, TQ], bf16, isOutput=False)
    if mode == "masked":
        maskT = nc.declare_dram_parameter("maskT", [S, S], f32, isOutput=False)
    out = nc.declare_dram_parameter("out", [TQ, D], f32, isOutput=True)

    with tile.TileContext(nc) as tc:
        from contextlib import ExitStack

        with (
            tc.tile_pool(name="dram", bufs=1, space="DRAM") as dram,
        ):
            a2a_in = [
                dram.tile([N_CORES, 128, TQ], bf16, name=f"a2a_in{h}")
                for h in range(NH_LOC)
            ]
            a2a_out = [
                dram.tile([N_CORES, 128, TQ], bf16, name=f"a2a_out{h}")
                for h in range(NH_LOC)
            ]

            actx = ExitStack()
            singles = actx.enter_context(tc.tile_pool(name="singles", bufs=1))
            kvp = actx.enter_context(tc.tile_pool(name="kvp", bufs=1))
            work = actx.enter_context(tc.tile_pool(name="work", bufs=3))
            psctx = ExitStack()
            pp = psctx.enter_context(tc.tile_pool(name="pp", bufs=2, space="PSUM"))
            pacc = psctx.enter_context(
                tc.tile_pool(name="pacc", bufs=2, space="PSUM")
            )
            psums = psctx.enter_context(
                tc.tile_pool(name="psums", bufs=2, space="PSUM")
            )
            xtctx = ExitStack()
            xtp = xtctx.enter_context(tc.tile_pool(name="xtp", bufs=2))

            # ---- resident weights/constants, load order = first-use order:
            # wk + slot-0 activations first so the PE starts ~immediately,
            # then wv (needed ~7us in), wq (needed ~14us in), then the rest.
            def load_xt(j):
                # host pre-tiled per slot: per-partition contiguous 32KB
                xt_t = xtp.tile([128, NKC, TQ], bf16, tag="xt", name=f"xt{j}")
                for q2 in range(2):
                    nc.sync.dma_start(
                        xt_t[:, q2 * 16 : (q2 + 1) * 16, :],
                        xT[j, :, q2 * 16 : (q2 + 1) * 16, :],
                    )
                cos_sl = work.tile([64, TQ], f32, tag="cos", bufs=2, name=f"cos{j}")
                nc.sync.dma_start(cos_sl[:], cosT[:, j * TQ : (j + 1) * TQ])
                sin_sl = work.tile([64, TQ], f32, tag="sin", bufs=2, name=f"sin{j}")
                nc.sync.dma_start(sin_sl[:], sinT[:, j * TQ : (j + 1) * TQ])
                return xt_t, cos_sl, sin_sl

            wk_sb = singles.tile([128, NKC, HD], bf16)
            for half in range(2):
                nc.sync.dma_start(
                    wk_sb[:, half * 16 : (half + 1) * 16, :],
                    wk[:, half * 16 : (half + 1) * 16, :],
                )
            xt0 = load_xt(0)
            wv_sb = singles.tile([128, NKC, HD], bf16)
            nc.gpsimd.dma_start(wv_sb[:], wv[:, :, :])
            wq_sb = xtp.tile([128, NKC, NH_LOC * HD], bf16, bufs=1)
            for half in range(2):
                nc.scalar.dma_start(
                    wq_sb[:, half * 16 : (half + 1) * 16, :],
                    wq[:, half * 16 : (half + 1) * 16, :],
                )
            # all-ones stationary: ones^T @ P^T replicates the per-token key-sum
            # across all 128 PSUM partitions (avoids partition-broadcast later)
            ones_sb = singles.tile([128, 128], bf16)
            nc.vector.memset(ones_sb, 1.0)
            ident_sb = singles.tile([128, 128], bf16)
            make_identity(nc, ident_sb)
            if mode == "causal":
                mask4_sb = singles.tile([128, 4, TQ], bf16)
                nc.scalar.dma_start(mask4_sb[:], mask4[:, :, :])

            # resident K^T [hd, tok], V natural [tk, kb, hd]; Q^T spills to
            # DRAM (tiny contiguous reloads) to keep SBUF headroom
            kres = kvp.tile([128, TOK], bf16)
            vres = kvp.tile([128, TOK // TKB, HD], bf16)
            qres = dram.tile([NH_LOC, NSLOT, 128, TQ], bf16)

            def rope(dst, ps, cos_sl, sin_sl):
                """dst[hd, t] (bf16) <- rotate(ps[hd, t]) with de-interleaved
                halves: rows 0:64 = even lanes (t0), 64:128 = odd lanes (t1).
                out0 = t0*c - t1*s ; out1 = t0*s + t1*c."""
                t0, t1 = ps[0:64], ps[64:128]
                ta = work.tile([64, TQ], f32, tag="rope_a", bufs=2)
                tb = work.tile([64, TQ], f32, tag="rope_b", bufs=2)
                nc.vector.tensor_mul(ta[:], t0, cos_sl[:])
                nc.vector.tensor_mul(tb[:], t1, sin_sl[:])
                nc.vector.tensor_sub(dst[0:64], ta[:], tb[:])
                tc_ = work.tile([64, TQ], f32, tag="rope_a", bufs=2)
                td = work.tile([64, TQ], f32, tag="rope_b", bufs=2)
                nc.vector.tensor_mul(tc_[:], t0, sin_sl[:])
                nc.vector.tensor_mul(td[:], t1, cos_sl[:])
                nc.vector.tensor_add(dst[64:128], tc_[:], td[:])

            # ---- phase A1: all projections, one 512-token slot at a time ----
            for j in range(NSLOT):
                xt_t, cos_sl, sin_sl = xt0 if j == 0 else load_xt(j)

                # K^T
                ps_k = pp.tile([128, TQ], f32, tag="mm")
                for c in range(NKC):
                    nc.tensor.matmul(
                        ps_k[:], wk_sb[:, c, :], xt_t[:, c, :],
                        start=(c == 0), stop=(c == NKC - 1),
                    )
                rope(kres[:, j * TQ : (j + 1) * TQ], ps_k, cos_sl, sin_sl)

                # V^T (transposed to natural after Qproj h=0 fills the PE)
                ps_v = pp.tile([128, TQ], f32, tag="mm")
                for c in range(NKC):
                    nc.tensor.matmul(
                        ps_v[:], wv_sb[:, c, :], xt_t[:, c, :],
                        start=(c == 0), stop=(c == NKC - 1),
                    )
                vt_sb = work.tile([128, TQ], bf16, tag="vt")
                nc.any.tensor_copy(out=vt_sb[:], in_=ps_v[:])

                ps_q = pp.tile([128, TQ], f32, tag="mm")
                for c in range(NKC):
                    nc.tensor.matmul(
                        ps_q[:], wq_sb[:, c, 0:HD], xt_t[:, c, :],
                        start=(c == 0), stop=(c == NKC - 1),
                    )
                qw = work.tile([128, TQ], bf16, tag="qw", bufs=2, name=f"qw{j}_0")
                rope(qw[:], ps_q, cos_sl, sin_sl)
                nc.scalar.dma_start(qres[0, j], qw[:])

                ps_tr = pp.tile([128, TQ], bf16, tag="mm")
                for t in range(TQ // 128):
                    nc.tensor.transpose(
                        ps_tr[:, t * 128 : (t + 1) * 128],
                        vt_sb[:, t * 128 : (t + 1) * 128],
                        ident_sb[:],
                    )
                nc.any.tensor_copy(
                    out=vres[:, j * 4 : j * 4 + 4, :], in_=ps_tr[:]
                )

                for h in range(1, NH_LOC):
                    ps_q = pp.tile([128, TQ], f32, tag="mm", name=f"ps_q{j}_{h}")
                    for c in range(NKC):
                        nc.tensor.matmul(
                            ps_q[:], wq_sb[:, c, h * HD : (h + 1) * HD],
                            xt_t[:, c, :],
                            start=(c == 0), stop=(c == NKC - 1),
                        )
                    qw = work.tile([128, TQ], bf16, tag="qw", bufs=2,
                                   name=f"qw{j}_{h}")
                    rope(qw[:], ps_q, cos_sl, sin_sl)
                    nc.scalar.dma_start(qres[h, j], qw[:])

            # ---- phase A2: attention, head-outer; fire A2A per head ----
            xtctx.close()
            aoctx = ExitStack()
            aop = aoctx.enter_context(tc.tile_pool(name="aop", bufs=1))
            ao_sb = []
            for h in range(NH_LOC):
                for b in range(B):
                    for qb in range(NQB):
                        j = b * NQB + qb
                        q_sl = work.tile([128, TQ], bf16, tag="q", bufs=2,
                                         name=f"q{h}_{j}")
                        nc.sync.dma_start(q_sl[:], qres[h, j])
                        q_sl = q_sl[:]
                        nkb = 4 * qb + 4 if mode == "causal" else 4 * NQB
                        kb0 = b * (S // TKB)  # K/V block offset for this batch

                        ps_o = pacc.tile([128, TQ], f32, tag="acc")
                        ps_sum = psums.tile([128, TQ], f32, tag="sums")

                        def scores_pair(pi, j=j, q_sl=q_sl, kb0=kb0, qb=qb):
                            """one exp for two key blocks: [128, 2, TQ]"""
                            ps_s = pp.tile([128, 2, TQ], f32, tag="mm",
                                           name=f"ps_s{j}_{pi}")
                            for i in range(2):
                                kk = kb0 + 2 * pi + i
                                nc.tensor.matmul(
                                    ps_s[:, i, :],
                                    kres[:, kk * TKB : (kk + 1) * TKB],
                                    q_sl, start=True, stop=True,
                                )
                            p_sb = work.tile([128, 2, TQ], bf16, tag="p",
                                             name=f"p_sb{j}_{pi}")
                            if mode == "masked":
                                smask = work.tile([128, 2, TQ], f32, tag="smask")
                                for i in range(2):
                                    kb = 2 * pi + i
                                    nc.sync.dma_start(
                                        smask[:, i, :],
                                        maskT[kb * TKB : (kb + 1) * TKB,
                                              qb * TQ : (qb + 1) * TQ],
                                    )
                                tmp_s = work.tile([128, 2, TQ], f32, tag="tmps")
                                nc.vector.tensor_scalar_mul(tmp_s[:], ps_s[:], SCALE)
                                nc.vector.tensor_add(tmp_s[:], tmp_s[:], smask[:])
                                nc.scalar.activation(
                                    p_sb[:], tmp_s[:],
                                    mybir.ActivationFunctionType.Exp,
                                )
                            else:
                                nc.scalar.activation(
                                    p_sb[:], ps_s[:],
                                    mybir.ActivationFunctionType.Exp,
                                    scale=SCALE,
                                )
                                if mode == "causal" and 2 * pi >= 4 * qb:
                                    nc.vector.tensor_mul(
                                        p_sb[:], p_sb[:],
                                        mask4_sb[:, 2 * (pi - 2 * qb) :
                                                 2 * (pi - 2 * qb) + 2, :],
                                    )
                            return p_sb

                        # prologue: one pair tile, but TWO separate exps
                        # (each ready ~400ns sooner than a joint 1024-wide exp)
                        ps_s01 = pp.tile([128, 2, TQ], f32, tag="mm",
                                         name=f"ps_g{j}")
                        p_sb01 = work.tile([128, 2, TQ], bf16, tag="p",
                                           name=f"p_g{j}")
                        for i in range(2):
                            nc.tensor.matmul(
                                ps_s01[:, i, :],
                                kres[:, (kb0 + i) * TKB : (kb0 + i + 1) * TKB],
                                q_sl, start=True, stop=True,
                            )
                        for i in range(2):
                            if mode == "masked":
                                smask = work.tile([128, 2, TQ], f32, tag="smask",
                                                  name=f"smg{j}_{i}")
                                nc.sync.dma_start(
                                    smask[:, 0, :],
                                    maskT[i * TKB : (i + 1) * TKB,
                                          qb * TQ : (qb + 1) * TQ],
                                )
                                tmp_s = work.tile([128, 2, TQ], f32, tag="tmps",
                                                  name=f"tmg{j}_{i}")
                                nc.vector.tensor_scalar_mul(
                                    tmp_s[:, 0, :], ps_s01[:, i, :], SCALE
                                )
                                nc.vector.tensor_add(
                                    tmp_s[:, 0, :], tmp_s[:, 0, :], smask[:, 0, :]
                                )
                                nc.scalar.activation(
                                    p_sb01[:, i, :], tmp_s[:, 0, :],
                                    mybir.ActivationFunctionType.Exp,
                                )
                            else:
                                nc.scalar.activation(
                                    p_sb01[:, i, :], ps_s01[:, i, :],
                                    mybir.ActivationFunctionType.Exp,
                                    scale=SCALE,
                                )
                                if mode == "causal" and i >= 4 * qb:
                                    nc.vector.tensor_mul(
                                        p_sb01[:, i, :], p_sb01[:, i, :],
                                        mask4_sb[:, i - 4 * qb, :],
                                    )

                        npairs = nkb // 2
                        p_cur = scores_pair(1)
                        for i in range(2):
                            nc.tensor.matmul(
                                ps_o[:], vres[:, kb0 + i, :], p_sb01[:, i, :],
                                start=(i == 0), stop=False,
                            )
                            nc.tensor.matmul(
                                ps_sum[:], ones_sb[:], p_sb01[:, i, :],
                                start=(i == 0), stop=False,
                            )
                        for pi in range(1, npairs):
                            p_next = scores_pair(pi + 1) if pi + 1 < npairs else None
                            for i in range(2):
                                kb = 2 * pi + i
                                kk = kb0 + kb
                                nc.tensor.matmul(
                                    ps_o[:], vres[:, kk, :], p_cur[:, i, :],
                                    start=False, stop=(kb == nkb - 1),
                                )
                                nc.tensor.matmul(
                                    ps_sum[:], ones_sb[:], p_cur[:, i, :],
                                    start=False, stop=(kb == nkb - 1),
                                )
                            p_cur = p_next

                        recip = work.tile([128, TQ], f32, tag="recip", bufs=2)
                        nc.vector.reciprocal_approx_fast(recip[:], ps_sum[:])
                        o_sb = work.tile([128, TQ], bf16, tag="o", bufs=2)
                        nc.vector.tensor_mul(o_sb[:], ps_o[:], recip[:])
                        nc.sync.dma_start(a2a_in[h][j], o_sb[:])

                nc.gpsimd.collective_compute(
                    "AllToAll",
                    mybir.AluOpType.bypass,
                    replica_groups=[list(range(N_CORES))],
                    ins=[a2a_in[h].opt()],
                    outs=[a2a_out[h].opt()],
                )
                ao_t = aop.tile([128, N_CORES, TQ], bf16, name=f"ao_sb{h}")
                for s_ in range(N_CORES):
                    # gpsimd: its queue only carries the (serial) collectives
                    # in A2, so blocking on collective h is harmless here
                    nc.gpsimd.dma_start(ao_t[:, s_, :], a2a_out[h][s_])
                ao_sb.append(ao_t)

            # ---- phase C: out[my 512 tokens] = AO @ wo (full wo) ----
            with (
                tc.tile_pool(name="wop", bufs=2) as wop,
                tc.tile_pool(name="outp", bufs=3) as outp,
            ):
                for nb in range(D // TQ):
                    wo_t = wop.tile([128, NH, TQ], bf16, tag="wot")
                    for q2 in range(2):
                        nc.sync.dma_start(
                            wo_t[:, q2 * 16 : (q2 + 1) * 16, :],
                            wo[nb, :, q2 * 16 : (q2 + 1) * 16, :],
                        )
                    pair_a = pp.tile([128, 2, TQ], f32, tag="mm",
                                     name=f"ps_outa{nb}")
                    pair_b = pp.tile([128, 2, TQ], f32, tag="mm",
                                     name=f"ps_outb{nb}")
                    ps_out = [pair_a[:, 0, :], pair_a[:, 1, :],
                              pair_b[:, 0, :], pair_b[:, 1, :]]
                    first = True
                    for hg in range(NH_LOC):
                        for s_ in range(N_CORES):
                            k = 4 * s_ + hg
                            last = hg == NH_LOC - 1 and s_ == N_CORES - 1
                            for m in range(4):
                                nc.tensor.matmul(
                                    ps_out[m],
                                    ao_sb[hg][:, s_, m * 128 : (m + 1) * 128],
                                    wo_t[:, k, :],
                                    start=first, stop=last,
                                )
                            first = False
                    for m in range(4):
                        osb = outp.tile([128, TQ], f32, tag="osb")
                        nc.any.tensor_copy(out=osb[:], in_=ps_out[m])
                        nc.scalar.dma_start(
                            out[m * 128 : (m + 1) * 128, nb * TQ : (nb + 1) * TQ],
                            osb[:],
                        )
            aoctx.close()
            psctx.close()
            actx.close()

    nc.finalize()
    return nc


def _detect_mode(mask: np.ndarray) -> str:
    if not np.any(mask):
        return "dense"
    tril_ok = not np.any(mask[np.tril_indices(S)])
    iu = np.triu_indices(S, 1)
    triu_ok = np.all(mask[iu] <= -1e8)
    if tril_ok and triu_ok:
        return "causal"
    return "masked"


def kernel(x, wq, wk, wv, wo, cache_k, cache_v, freqs_cos, freqs_sin, mask,
           start_pos):
    from ml_dtypes import bfloat16

    from concourse.bass_utils import run_bass_kernel_spmd

    assert int(start_pos) == 0, "kernel hardcodes start_pos == 0"
    x = np.asarray(x, dtype=np.float32)
    wq = np.asarray(wq, dtype=np.float32)
    wk = np.asarray(wk, dtype=np.float32)
    wv = np.asarray(wv, dtype=np.float32)
    wo = np.asarray(wo, dtype=np.float32)
    freqs_cos = np.asarray(freqs_cos, dtype=np.float32)
    freqs_sin = np.asarray(freqs_sin, dtype=np.float32)
    mask = np.asarray(mask, dtype=np.float32)

    mode = _detect_mode(mask)
    if mode not in _NC_CACHE:
        _NC_CACHE[mode] = _build_nc(mode)
    nc = _NC_CACHE[mode]

    # X^T slot-tiled [8, 128, 32, 512]: [j, p, c, t] = x_flat[512j+t, 128c+p]
    x_flat = x.reshape(TOK, D)
    xT = np.ascontiguousarray(
        x_flat.T.reshape(NKC, 128, NSLOT, TQ).transpose(2, 1, 0, 3)
    ).astype(bfloat16)

    # de-interleave RoPE pairs within each head: [0,2,...,126,1,3,...,127]
    perm = np.concatenate([np.arange(0, HD, 2), np.arange(1, HD, 2)])

    # cos/sin transposed, tiled over batches: [64, 4096]
    cosT = np.ascontiguousarray(
        np.concatenate([freqs_cos.T] * B, axis=1), dtype=np.float32
    )
    sinT = np.ascontiguousarray(
        np.concatenate([freqs_sin.T] * B, axis=1), dtype=np.float32
    )

    # wo nb-tiled [8, 128, 32, 512]: [nb, p, k, n] = wo[128k+p, 512nb+n]
    wo_bf = np.ascontiguousarray(
        wo.reshape(NH, 128, D // TQ, TQ).transpose(2, 1, 0, 3)
    ).astype(bfloat16)

    def to_chunked(w):  # [4096, F] -> [128, 32, F]
        return np.ascontiguousarray(
            w.reshape(NKC, 128, w.shape[1]).transpose(1, 0, 2)
        ).astype(bfloat16)

    if mode == "causal":
        # mask4[p, c, t] = 1 if t >= 128c + p else 0  (multiplicative, bf16)
        t_idx = np.arange(TQ)[None, None, :]
        p_idx = np.arange(128)[:, None, None]
        c_idx = np.arange(4)[None, :, None]
        mask4 = (t_idx >= 128 * c_idx + p_idx).astype(bfloat16)

    in_maps = []
    for r in range(N_CORES):
        q_cols = np.concatenate(
            [(4 * r + h) * HD + perm for h in range(NH_LOC)]
        )
        m = {
            "xT": xT,
            "wq": to_chunked(wq[:, q_cols]),
            "wk": to_chunked(wk[:, r * HD + perm]),
            "wv": to_chunked(wv[:, r * HD : (r + 1) * HD]),
            "wo": wo_bf,
            "cosT": cosT,
            "sinT": sinT,
        }
        if mode == "causal":
            m["mask4"] = mask4
        if mode == "masked":
            m["maskT"] = np.ascontiguousarray(mask.T)
        in_maps.append(m)

    kwargs = {}
    if PROFILE and TRACE_DIR is not None:
        kwargs["tmpdir"] = TRACE_DIR
    res = run_bass_kernel_spmd(
        nc, in_maps, list(range(N_CORES)), trace=PROFILE, **kwargs
    )
    global LAST_EXEC_NS, LAST_TRACE_DIR
    LAST_EXEC_NS = res.exec_time_ns
    if PROFILE and res.profile_json is not None:
        LAST_TRACE_DIR = res.profile_json

    out_full = np.empty((TOK, D), dtype=np.float32)
    for r in range(N_CORES):
        out_full[r * TQ : (r + 1) * TQ] = res.results[r]["out"]
    return out_full.reshape(B, S, D)


# revision 35
# speedup vs baseline: 1.0202x; 1.0202x over previous
"""Trainium2 8-core tensor-parallel attention kernel (Bass/Tile).

Strategy (TP over heads, per the ColumnParallel/RowParallel intent):
  - Each of the 8 cores owns 1 KV head and its 4 GQA query heads.
  - Phase A1: Q/K/V projections for all 8 512-token slots (bf16 matmuls,
    fp32 PSUM), RoPE on-chip; Q^T/K^T/V kept SBUF-resident.
  - Phase A2: causal attention, head-outer order, no max-subtraction
    (scores are bounded so exp is safe in fp32); O^T = V^T P^T accumulated
    in PSUM over key blocks; per-token softmax denominators via an
    all-ones stationary matmul (replicated across partitions). The key-
    block loop is software-pipelined one deep so the ACT-engine exp never
    stalls the TensorEngine. After each head finishes, a per-head
    AllToAll fires, overlapping later heads' attention.
  - Phase C: the output projection is sequence-parallel: the AllToAlls
    convert head-sharding to token-sharding, then each core multiplies
    its 512-token slice by the FULL wo. No all-reduce; the host
    concatenates the 8 disjoint token slices.

Layout choices:
  - Activations stay transposed (X^T/Q^T/K^T/O^T: features on the 128
    partitions, tokens on the free axis) so every matmul streams 512 wide
    and fp32 DMA-transposes are never needed; V is transposed to natural
    token-major via TensorE transpose (cheap).
  - RoPE pairs are de-interleaved on the host by permuting wq/wk columns
    (even lanes then odd lanes within each head) -> RoPE is 6 vector ops
    on partition halves. The permutation cancels in Q.K^T.
  - All matmul operands are bf16 (fp32 PSUM accumulation).
"""

import math

import numpy as np

B, S, D = 2, 2048, 4096
NH, NKV, HD = 32, 8, 128
N_REP = NH // NKV
N_CORES = 8
TOK = B * S            # 4096 flattened tokens
TQ = 512               # query-block width (matmul moving free dim)
TKB = 128              # key-block width (stationary free dim)
NKC = D // 128         # 32 contraction chunks of 128
NQB = S // TQ          # 4 query blocks per batch
NSLOT = B * NQB        # 8 512-token slots
NH_LOC = NH // N_CORES  # 4 query heads per core
SCALE = 1.0 / math.sqrt(HD)

PROFILE = False
TRACE_DIR = None
LAST_EXEC_NS = None
LAST_TRACE_DIR = None

_NC_CACHE = {}


def _build_nc(mode: str):
    """mode: 'causal' (skip fully-masked blocks, triangular diag masks),
    'dense' (no mask at all), 'masked' (generic additive mask from DRAM)."""
    import concourse.tile as tile
    from concourse import bacc, mybir
    from concourse.masks import make_identity

    f32 = mybir.dt.float32
    bf16 = mybir.dt.bfloat16

    nc = bacc.Bacc(None, target_bir_lowering=False, num_devices=N_CORES)

    xT = nc.declare_dram_parameter("xT", [NSLOT, 128, NKC, TQ], bf16, isOutput=False)
    wq = nc.declare_dram_parameter("wq", [128, NKC, NH_LOC * HD], bf16, isOutput=False)
    wk = nc.declare_dram_parameter("wk", [128, NKC, HD], bf16, isOutput=False)
    wv = nc.declare_dram_parameter("wv", [128, NKC, HD], bf16, isOutput=False)
    wo = nc.declare_dram_parameter("wo", [D // TQ, 128, NH, TQ], bf16, isOutput=False)
    cosT = nc.declare_dram_parameter("cosT", [64, TOK], f32, isOutput=False)
    sinT = nc.declare_dram_parameter("sinT", [64, TOK], f32, isOutput=False)
    if mode == "causal":
        mask4 = nc.declare_dram_parameter("mask4", [128, 4, TQ], bf16, isOutput=False)
    if mode == "masked":
        maskT = nc.declare_dram_parameter("maskT", [S, S], f32, isOutput=False)
    out = nc.declare_dram_parameter("out", [TQ, D], f32, isOutput=True)

    with tile.TileContext(nc) as tc:
        from contextlib import ExitStack

        with (
            tc.tile_pool(name="dram", bufs=1, space="DRAM") as dram,
        ):
            a2a_in = [
                dram.tile([N_CORES, 128, TQ], bf16, name=f"a2a_in{h}")
                for h in range(NH_LOC)
            ]
            a2a_out = [
                dram.tile([N_CORES, 128, TQ], bf16, name=f"a2a_out{h}")
                for h in range(NH_LOC)
            ]

            actx = ExitStack()
            singles = actx.enter_context(tc.tile_pool(name="singles", bufs=1))
            kvp = actx.enter_context(tc.tile_pool(name="kvp", bufs=1))
            work = actx.enter_context(tc.tile_pool(name="work", bufs=3))
            psctx = ExitStack()
            pp = psctx.enter_context(tc.tile_pool(name="pp", bufs=2, space="PSUM"))
            pacc = psctx.enter_context(
                tc.tile_pool(name="pacc", bufs=2, space="PSUM")
            )
            psums = psctx.enter_context(
                tc.tile_pool(name="psums", bufs=2, space="PSUM")
            )
            xtctx = ExitStack()
            xtp = xtctx.enter_context(tc.tile_pool(name="xtp", bufs=2))

            # ---- resident weights/constants, load order = first-use order:
            # wk + slot-0 activations first so the PE starts ~immediately,
            # then wv (needed ~7us in), wq (needed ~14us in), then the rest.
            def load_xt(j):
                # host pre-tiled per slot: per-partition contiguous 32KB
                xt_t = xtp.tile([128, NKC, TQ], bf16, tag="xt", name=f"xt{j}")
                for q4 in range(4):  # 4 queues in parallel
                    nc.sync.dma_start(
                        xt_t[:, q4 * 8 : (q4 + 1) * 8, :],
                        xT[j, :, q4 * 8 : (q4 + 1) * 8, :],
                    )
                cos_sl = work.tile([64, TQ], f32, tag="cos", bufs=2, name=f"cos{j}")
                nc.sync.dma_start(cos_sl[:], cosT[:, j * TQ : (j + 1) * TQ])
                sin_sl = work.tile([64, TQ], f32, tag="sin", bufs=2, name=f"sin{j}")
                nc.sync.dma_start(sin_sl[:], sinT[:, j * TQ : (j + 1) * TQ])
                return xt_t, cos_sl, sin_sl

            wk_sb = singles.tile([128, NKC, HD], bf16)
            for half in range(2):
                nc.sync.dma_start(
                    wk_sb[:, half * 16 : (half + 1) * 16, :],
                    wk[:, half * 16 : (half + 1) * 16, :],
                )
            xt0 = load_xt(0)
            wv_sb = singles.tile([128, NKC, HD], bf16)
            nc.gpsimd.dma_start(wv_sb[:], wv[:, :, :])
            wq_sb = xtp.tile([128, NKC, NH_LOC * HD], bf16, bufs=1)
            for half in range(2):
                nc.scalar.dma_start(
                    wq_sb[:, half * 16 : (half + 1) * 16, :],
                    wq[:, half * 16 : (half + 1) * 16, :],
                )
            # all-ones stationary: ones^T @ P^T replicates the per-token key-sum
            # across all 128 PSUM partitions (avoids partition-broadcast later)
            ones_sb = singles.tile([128, 128], bf16)
            nc.vector.memset(ones_sb, 1.0)
            ident_sb = singles.tile([128, 128], bf16)
            make_identity(nc, ident_sb)
            if mode == "causal":
                mask4_sb = singles.tile([128, 4, TQ], bf16)
                nc.scalar.dma_start(mask4_sb[:], mask4[:, :, :])

            # resident K^T [hd, tok], V natural [tk, kb, hd]; Q^T spills to
            # DRAM (tiny contiguous reloads) to keep SBUF headroom
            kres = kvp.tile([128, TOK], bf16)
            vres = kvp.tile([128, TOK // TKB, HD], bf16)
            qres = dram.tile([NH_LOC, NSLOT, 128, TQ], bf16)

            def rope(dst, ps, cos_sl, sin_sl):
                """dst[hd, t] (bf16) <- rotate(ps[hd, t]) with de-interleaved
                halves: rows 0:64 = even lanes (t0), 64:128 = odd lanes (t1).
                out0 = t0*c - t1*s ; out1 = t0*s + t1*c."""
                t0, t1 = ps[0:64], ps[64:128]
                ta = work.tile([64, TQ], f32, tag="rope_a", bufs=2)
                tb = work.tile([64, TQ], f32, tag="rope_b", bufs=2)
                nc.vector.tensor_mul(ta[:], t0, cos_sl[:])
                nc.vector.tensor_mul(tb[:], t1, sin_sl[:])
                nc.vector.tensor_sub(dst[0:64], ta[:], tb[:])
                tc_ = work.tile([64, TQ], f32, tag="rope_a", bufs=2)
                td = work.tile([64, TQ], f32, tag="rope_b", bufs=2)
                nc.vector.tensor_mul(tc_[:], t0, sin_sl[:])
                nc.vector.tensor_mul(td[:], t1, cos_sl[:])
                nc.vector.tensor_add(dst[64:128], tc_[:], td[:])

            # ---- phase A1: all projections, one 512-token slot at a time ----
            for j in range(NSLOT):
                xt_t, cos_sl, sin_sl = xt0 if j == 0 else load_xt(j)

                # K^T
                ps_k = pp.tile([128, TQ], f32, tag="mm")
                for c in range(NKC):
                    nc.tensor.matmul(
                        ps_k[:], wk_sb[:, c, :], xt_t[:, c, :],
                        start=(c == 0), stop=(c == NKC - 1),
                    )
                rope(kres[:, j * TQ : (j + 1) * TQ], ps_k, cos_sl, sin_sl)

                # V^T (transposed to natural after Qproj h=0 fills the PE)
                ps_v = pp.tile([128, TQ], f32, tag="mm")
                for c in range(NKC):
                    nc.tensor.matmul(
                        ps_v[:], wv_sb[:, c, :], xt_t[:, c, :],
                        start=(c == 0), stop=(c == NKC - 1),
                    )
                vt_sb = work.tile([128, TQ], bf16, tag="vt")
                nc.any.tensor_copy(out=vt_sb[:], in_=ps_v[:])

                ps_q = pp.tile([128, TQ], f32, tag="mm")
                for c in range(NKC):
                    nc.tensor.matmul(
                        ps_q[:], wq_sb[:, c, 0:HD], xt_t[:, c, :],
                        start=(c == 0), stop=(c == NKC - 1),
                    )
                qw = work.tile([128, TQ], bf16, tag="qw", bufs=2, name=f"qw{j}_0")
                rope(qw[:], ps_q, cos_sl, sin_sl)
                nc.scalar.dma_start(qres[0, j], qw[:])

                ps_tr = pp.tile([128, TQ], bf16, tag="mm")
                for t in range(TQ // 128):
                    nc.tensor.transpose(
                        ps_tr[:, t * 128 : (t + 1) * 128],
                        vt_sb[:, t * 128 : (t + 1) * 128],
                        ident_sb[:],
                    )
                nc.any.tensor_copy(
                    out=vres[:, j * 4 : j * 4 + 4, :], in_=ps_tr[:]
                )

                for h in range(1, NH_LOC):
                    ps_q = pp.tile([128, TQ], f32, tag="mm", name=f"ps_q{j}_{h}")
                    for c in range(NKC):
                        nc.tensor.matmul(
                            ps_q[:], wq_sb[:, c, h * HD : (h + 1) * HD],
                            xt_t[:, c, :],
                            start=(c == 0), stop=(c == NKC - 1),
                        )
                    qw = work.tile([128, TQ], bf16, tag="qw", bufs=2,
                                   name=f"qw{j}_{h}")
                    rope(qw[:], ps_q, cos_sl, sin_sl)
                    nc.scalar.dma_start(qres[h, j], qw[:])

            # ---- phase A2: attention, head-outer; fire A2A per head ----
            xtctx.close()
            aoctx = ExitStack()
            aop = aoctx.enter_context(tc.tile_pool(name="aop", bufs=1))
            ao_sb = []
            for h in range(NH_LOC):
                for b in range(B):
                    for qb in range(NQB):
                        j = b * NQB + qb
                        q_sl = work.tile([128, TQ], bf16, tag="q", bufs=2,
                                         name=f"q{h}_{j}")
                        nc.sync.dma_start(q_sl[:], qres[h, j])
                        q_sl = q_sl[:]
                        nkb = 4 * qb + 4 if mode == "causal" else 4 * NQB
                        kb0 = b * (S // TKB)  # K/V block offset for this batch

                        ps_o = pacc.tile([128, TQ], f32, tag="acc")
                        ps_sum = psums.tile([128, TQ], f32, tag="sums")

                        def scores_pair(pi, j=j, q_sl=q_sl, kb0=kb0, qb=qb):
                            """one exp for two key blocks: [128, 2, TQ]"""
                            ps_s = pp.tile([128, 2, TQ], f32, tag="mm",
                                           name=f"ps_s{j}_{pi}")
                            for i in range(2):
                                kk = kb0 + 2 * pi + i
                                nc.tensor.matmul(
                                    ps_s[:, i, :],
                                    kres[:, kk * TKB : (kk + 1) * TKB],
                                    q_sl, start=True, stop=True,
                                )
                            p_sb = work.tile([128, 2, TQ], bf16, tag="p",
                                             name=f"p_sb{j}_{pi}")
                            if mode == "masked":
                                smask = work.tile([128, 2, TQ], f32, tag="smask")
                                for i in range(2):
                                    kb = 2 * pi + i
                                    nc.sync.dma_start(
                                        smask[:, i, :],
                                        maskT[kb * TKB : (kb + 1) * TKB,
                                              qb * TQ : (qb + 1) * TQ],
                                    )
                                tmp_s = work.tile([128, 2, TQ], f32, tag="tmps")
                                nc.vector.tensor_scalar_mul(tmp_s[:], ps_s[:], SCALE)
                                nc.vector.tensor_add(tmp_s[:], tmp_s[:], smask[:])
                                nc.scalar.activation(
                                    p_sb[:], tmp_s[:],
                                    mybir.ActivationFunctionType.Exp,
                                )
                            else:
                                nc.scalar.activation(
                                    p_sb[:], ps_s[:],
                                    mybir.ActivationFunctionType.Exp,
                                    scale=SCALE,
                                )
                                if mode == "causal" and 2 * pi >= 4 * qb:
                                    nc.vector.tensor_mul(
                                        p_sb[:], p_sb[:],
                                        mask4_sb[:, 2 * (pi - 2 * qb) :
                                                 2 * (pi - 2 * qb) + 2, :],
                                    )
                            return p_sb

                        # prologue: one pair tile, but TWO separate exps
                        # (each ready ~400ns sooner than a joint 1024-wide exp)
                        ps_s01 = pp.tile([128, 2, TQ], f32, tag="mm",
                                         name=f"ps_g{j}")
                        p_sb01 = work.tile([128, 2, TQ], bf16, tag="p",
                                           name=f"p_g{j}")
                        for i in range(2):
                            nc.tensor.matmul(
                                ps_s01[:, i, :],
                                kres[:, (kb0 + i) * TKB : (kb0 + i + 1) * TKB],
                                q_sl, start=True, stop=True,
                            )
                        for i in range(2):
                            if mode == "masked":
                                smask = work.tile([128, 2, TQ], f32, tag="smask",
                                                  name=f"smg{j}_{i}")
                                nc.sync.dma_start(
                                    smask[:, 0, :],
                                    maskT[i * TKB : (i + 1) * TKB,
                                          qb * TQ : (qb + 1) * TQ],
                                )
                                tmp_s = work.tile([128, 2, TQ], f32, tag="tmps",
                                                  name=f"tmg{j}_{i}")
                                nc.vector.tensor_scalar_mul(
                                    tmp_s[:, 0, :], ps_s01[:, i, :], SCALE
                                )
                                nc.vector.tensor_add(
                                    tmp_s[:, 0, :], tmp_s[:, 0, :], smask[:, 0, :]
                                )
                                nc.scalar.activation(
                                    p_sb01[:, i, :], tmp_s[:, 0, :],
                                    mybir.ActivationFunctionType.Exp,
                                )
                            else:
                                nc.scalar.activation(
                                    p_sb01[:, i, :], ps_s01[:, i, :],
                                    mybir.ActivationFunctionType.Exp,
                                    scale=SCALE,
                                )
                                if mode == "causal" and i >= 4 * qb:
                                    nc.vector.tensor_mul(
                                        p_sb01[:, i, :], p_sb01[:, i, :],
                                        mask4_sb[:, i - 4 * qb, :],
                                    )

                        npairs = nkb // 2
                        p_cur = scores_pair(1)
                        for i in range(2):
                            nc.tensor.matmul(
                                ps_o[:], vres[:, kb0 + i, :], p_sb01[:, i, :],
                                start=(i == 0), stop=False,
                            )
                            nc.tensor.matmul(
                                ps_sum[:], ones_sb[:], p_sb01[:, i, :],
                                start=(i == 0), stop=False,
                            )
                        for pi in range(1, npairs):
                            p_next = scores_pair(pi + 1) if pi + 1 < npairs else None
                            for i in range(2):
                                kb = 2 * pi + i
                                kk = kb0 + kb
                                nc.tensor.matmul(
                                    ps_o[:], vres[:, kk, :], p_cur[:, i, :],
                                    start=False, stop=(kb == nkb - 1),
                                )
                                nc.tensor.matmul(
                                    ps_sum[:], ones_sb[:], p_cur[:, i, :],
                                    start=False, stop=(kb == nkb - 1),
                                )
                            p_cur = p_next

                        recip = work.tile([128, TQ], f32, tag="recip", bufs=2)
                        nc.vector.reciprocal_approx_fast(recip[:], ps_sum[:])
                        o_sb = work.tile([128, TQ], bf16, tag="o", bufs=2)
                        nc.vector.tensor_mul(o_sb[:], ps_o[:], recip[:])
                        nc.sync.dma_start(a2a_in[h][j], o_sb[:])

                nc.gpsimd.collective_compute(
                    "AllToAll",
                    mybir.AluOpType.bypass,
                    replica_groups=[list(range(N_CORES))],
                    ins=[a2a_in[h].opt()],
                    outs=[a2a_out[h].opt()],
                )
                ao_t = aop.tile([128, N_CORES, TQ], bf16, name=f"ao_sb{h}")
                for s_ in range(N_CORES):
                    # gpsimd: its queue only carries the (serial) collectives
                    # in A2, so blocking on collective h is harmless here
                    nc.gpsimd.dma_start(ao_t[:, s_, :], a2a_out[h][s_])
                ao_sb.append(ao_t)

            # ---- phase C: out[my 512 tokens] = AO @ wo (full wo) ----
            with (
                tc.tile_pool(name="wop", bufs=2) as wop,
                tc.tile_pool(name="outp", bufs=3) as outp,
            ):
                for nb in range(D // TQ):
                    wo_t = wop.tile([128, NH, TQ], bf16, tag="wot")
                    for q8 in range(8):  # 8 queues in parallel
                        nc.sync.dma_start(
                            wo_t[:, q8 * 4 : (q8 + 1) * 4, :],
                            wo[nb, :, q8 * 4 : (q8 + 1) * 4, :],
                        )
                    pair_a = pp.tile([128, 2, TQ], f32, tag="mm",
                                     name=f"ps_outa{nb}")
                    pair_b = pp.tile([128, 2, TQ], f32, tag="mm",
                                     name=f"ps_outb{nb}")
                    ps_out = [pair_a[:, 0, :], pair_a[:, 1, :],
                              pair_b[:, 0, :], pair_b[:, 1, :]]
                    first = True
                    for hg in range(NH_LOC):
                        for s_ in range(N_CORES):
                            k = 4 * s_ + hg
                            last = hg == NH_LOC - 1 and s_ == N_CORES - 1
                            for m in range(4):
                                nc.tensor.matmul(
                                    ps_out[m],
                                    ao_sb[hg][:, s_, m * 128 : (m + 1) * 128],
                                    wo_t[:, k, :],
                                    start=first, stop=last,
                                )
                            first = False
                    for m in range(4):
                        osb = outp.tile([128, TQ], f32, tag="osb")
                        nc.any.tensor_copy(out=osb[:], in_=ps_out[m])
                        nc.scalar.dma_start(
                            out[m * 128 : (m + 1) * 128, nb * TQ : (nb + 1) * TQ],
                            osb[:],
                        )
            aoctx.close()
            psctx.close()
            actx.close()

    nc.finalize()
    return nc


def _detect_mode(mask: np.ndarray) -> str:
    if not np.any(mask):
        return "dense"
    tril_ok = not np.any(mask[np.tril_indices(S)])
    iu = np.triu_indices(S, 1)
    triu_ok = np.all(mask[iu] <= -1e8)
    if tril_ok and triu_ok:
        return "causal"
    return "masked"


def kernel(x, wq, wk, wv, wo, cache_k, cache_v, freqs_cos, freqs_sin, mask,
           start_pos):
    from ml_dtypes import bfloat16

    from concourse.bass_utils import run_bass_kernel_spmd

    assert int(start_pos) == 0, "kernel hardcodes start_pos == 0"
    x = np.asarray(x, dtype=np.float32)
    wq = np.asarray(wq, dtype=np.float32)
    wk = np.asarray(wk, dtype=np.float32)
    wv = np.asarray(wv, dtype=np.float32)
    wo = np.asarray(wo, dtype=np.float32)
    freqs_cos = np.asarray(freqs_cos, dtype=np.float32)
    freqs_sin = np.asarray(freqs_sin, dtype=np.float32)
    mask = np.asarray(mask, dtype=np.float32)

    mode = _detect_mode(mask)
    if mode not in _NC_CACHE:
        _NC_CACHE[mode] = _build_nc(mode)
    nc = _NC_CACHE[mode]

    # X^T slot-tiled [8, 128, 32, 512]: [j, p, c, t] = x_flat[512j+t, 128c+p]
    x_flat = x.reshape(TOK, D)
    xT = np.ascontiguousarray(
        x_flat.T.reshape(NKC, 128, NSLOT, TQ).transpose(2, 1, 0, 3)
    ).astype(bfloat16)

    # de-interleave RoPE pairs within each head: [0,2,...,126,1,3,...,127]
    perm = np.concatenate([np.arange(0, HD, 2), np.arange(1, HD, 2)])

    # cos/sin transposed, tiled over batches: [64, 4096]
    cosT = np.ascontiguousarray(
        np.concatenate([freqs_cos.T] * B, axis=1), dtype=np.float32
    )
    sinT = np.ascontiguousarray(
        np.concatenate([freqs_sin.T] * B, axis=1), dtype=np.float32
    )

    # wo nb-tiled [8, 128, 32, 512]: [nb, p, k, n] = wo[128k+p, 512nb+n]
    wo_bf = np.ascontiguousarray(
        wo.reshape(NH, 128, D // TQ, TQ).transpose(2, 1, 0, 3)
    ).astype(bfloat16)

    def to_chunked(w):  # [4096, F] -> [128, 32, F]
        return np.ascontiguousarray(
            w.reshape(NKC, 128, w.shape[1]).transpose(1, 0, 2)
        ).astype(bfloat16)

    if mode == "causal":
        # mask4[p, c, t] = 1 if t >= 128c + p else 0  (multiplicative, bf16)
        t_idx = np.arange(TQ)[None, None, :]
        p_idx = np.arange(128)[:, None, None]
        c_idx = np.arange(4)[None, :, None]
        mask4 = (t_idx >= 128 * c_idx + p_idx).astype(bfloat16)

    in_maps = []
    for r in range(N_CORES):
        q_cols = np.concatenate(
            [(4 * r + h) * HD + perm for h in range(NH_LOC)]
        )
        m = {
            "xT": xT,
            "wq": to_chunked(wq[:, q_cols]),
            "wk": to_chunked(wk[:, r * HD + perm]),
            "wv": to_chunked(wv[:, r * HD : (r + 1) * HD]),
            "wo": wo_bf,
            "cosT": cosT,
            "sinT": sinT,
        }
        if mode == "causal":
            m["mask4"] = mask4
        if mode == "masked":
            m["maskT"] = np.ascontiguousarray(mask.T)
        in_maps.append(m)

    kwargs = {}
    if PROFILE and TRACE_DIR is not None:
        kwargs["tmpdir"] = TRACE_DIR
    res = run_bass_kernel_spmd(
        nc, in_maps, list(range(N_CORES)), trace=PROFILE, **kwargs
    )
    global LAST_EXEC_NS, LAST_TRACE_DIR
    LAST_EXEC_NS = res.exec_time_ns
    if PROFILE and res.profile_json is not None:
        LAST_TRACE_DIR = res.profile_json

    out_full = np.empty((TOK, D), dtype=np.float32)
    for r in range(N_CORES):
        out_full[r * TQ : (r + 1) * TQ] = res.results[r]["out"]
    return out_full.reshape(B, S, D)


# revision 36
# speedup vs baseline: 1.0565x; 1.0356x over previous
"""Trainium2 8-core tensor-parallel attention kernel (Bass/Tile).

Strategy (TP over heads, per the ColumnParallel/RowParallel intent):
  - Each of the 8 cores owns 1 KV head and its 4 GQA query heads.
  - Phase A1: Q/K/V projections for all 8 512-token slots (bf16 matmuls,
    fp32 PSUM), RoPE on-chip; Q^T/K^T/V kept SBUF-resident.
  - Phase A2: causal attention, head-outer order, no max-subtraction
    (scores are bounded so exp is safe in fp32); O^T = V^T P^T accumulated
    in PSUM over key blocks; per-token softmax denominators via an
    all-ones stationary matmul (replicated across partitions). The key-
    block loop is software-pipelined one deep so the ACT-engine exp never
    stalls the TensorEngine. After each head finishes, a per-head
    AllToAll fires, overlapping later heads' attention.
  - Phase C: the output projection is sequence-parallel: the AllToAlls
    convert head-sharding to token-sharding, then each core multiplies
    its 512-token slice by the FULL wo. No all-reduce; the host
    concatenates the 8 disjoint token slices.

Layout choices:
  - Activations stay transposed (X^T/Q^T/K^T/O^T: features on the 128
    partitions, tokens on the free axis) so every matmul streams 512 wide
    and fp32 DMA-transposes are never needed; V is transposed to natural
    token-major via TensorE transpose (cheap).
  - RoPE pairs are de-interleaved on the host by permuting wq/wk columns
    (even lanes then odd lanes within each head) -> RoPE is 6 vector ops
    on partition halves. The permutation cancels in Q.K^T.
  - All matmul operands are bf16 (fp32 PSUM accumulation).
"""

import math

import numpy as np

B, S, D = 2, 2048, 4096
NH, NKV, HD = 32, 8, 128
N_REP = NH // NKV
N_CORES = 8
TOK = B * S            # 4096 flattened tokens
TQ = 512               # query-block width (matmul moving free dim)
TKB = 128              # key-block width (stationary free dim)
NKC = D // 128         # 32 contraction chunks of 128
NQB = S // TQ          # 4 query blocks per batch
NSLOT = B * NQB        # 8 512-token slots
NH_LOC = NH // N_CORES  # 4 query heads per core
SCALE = 1.0 / math.sqrt(HD)

PROFILE = False
TRACE_DIR = None
LAST_EXEC_NS = None
LAST_TRACE_DIR = None

_NC_CACHE = {}


def _build_nc(mode: str):
    """mode: 'causal' (skip fully-masked blocks, triangular diag masks),
    'dense' (no mask at all), 'masked' (generic additive mask from DRAM)."""
    import concourse.tile as tile
    from concourse import bacc, mybir
    from concourse.masks import make_identity

    f32 = mybir.dt.float32
    bf16 = mybir.dt.bfloat16

    nc = bacc.Bacc(None, target_bir_lowering=False, num_devices=N_CORES)

    xT = nc.declare_dram_parameter("xT", [NSLOT, 128, NKC, TQ], bf16, isOutput=False)
    wq = nc.declare_dram_parameter("wq", [128, NKC, NH_LOC * HD], bf16, isOutput=False)
    wk = nc.declare_dram_parameter("wk", [128, NKC, HD], bf16, isOutput=False)
    wv = nc.declare_dram_parameter("wv", [128, NKC, HD], bf16, isOutput=False)
    wo = nc.declare_dram_parameter("wo", [D // TQ, 128, NH, TQ], bf16, isOutput=False)
    cosT = nc.declare_dram_parameter("cosT", [64, TOK], f32, isOutput=False)
    sinT = nc.declare_dram_parameter("sinT", [64, TOK], f32, isOutput=False)
    if mode == "causal":
        mask4 = nc.declare_dram_parameter("mask4", [128, 4, TQ], bf16, isOutput=False)
    if mode == "masked":
        maskT = nc.declare_dram_parameter("maskT", [S, S], f32, isOutput=False)
    out = nc.declare_dram_parameter("out", [TQ, D], f32, isOutput=True)

    with tile.TileContext(nc) as tc:
        from contextlib import ExitStack

        with (
            tc.tile_pool(name="dram", bufs=1, space="DRAM") as dram,
        ):
            a2a_in = [
                dram.tile([N_CORES, 128, TQ], bf16, name=f"a2a_in{h}")
                for h in range(NH_LOC)
            ]
            a2a_out = [
                dram.tile([N_CORES, 128, TQ], bf16, name=f"a2a_out{h}")
                for h in range(NH_LOC)
            ]

            actx = ExitStack()
            singles = actx.enter_context(tc.tile_pool(name="singles", bufs=1))
            kvp = actx.enter_context(tc.tile_pool(name="kvp", bufs=1))
            work = actx.enter_context(tc.tile_pool(name="work", bufs=3))
            psctx = ExitStack()
            pp = psctx.enter_context(tc.tile_pool(name="pp", bufs=2, space="PSUM"))
            pacc = psctx.enter_context(
                tc.tile_pool(name="pacc", bufs=2, space="PSUM")
            )
            psums = psctx.enter_context(
                tc.tile_pool(name="psums", bufs=2, space="PSUM")
            )
            xtctx = ExitStack()
            xtp = xtctx.enter_context(tc.tile_pool(name="xtp", bufs=2))

            # ---- resident weights/constants, load order = first-use order:
            # wk + slot-0 activations first so the PE starts ~immediately,
            # then wv (needed ~7us in), wq (needed ~14us in), then the rest.
            def load_xt(j):
                # host pre-tiled per slot: per-partition contiguous 32KB
                xt_t = xtp.tile([128, NKC, TQ], bf16, tag="xt", name=f"xt{j}")
                for q4 in range(4):  # 4 queues in parallel
                    nc.sync.dma_start(
                        xt_t[:, q4 * 8 : (q4 + 1) * 8, :],
                        xT[j, :, q4 * 8 : (q4 + 1) * 8, :],
                    )
                cos_sl = work.tile([64, TQ], f32, tag="cos", bufs=2, name=f"cos{j}")
                nc.sync.dma_start(cos_sl[:], cosT[:, j * TQ : (j + 1) * TQ])
                sin_sl = work.tile([64, TQ], f32, tag="sin", bufs=2, name=f"sin{j}")
                nc.sync.dma_start(sin_sl[:], sinT[:, j * TQ : (j + 1) * TQ])
                return xt_t, cos_sl, sin_sl

            wk_sb = singles.tile([128, NKC, HD], bf16)
            for half in range(2):
                nc.sync.dma_start(
                    wk_sb[:, half * 16 : (half + 1) * 16, :],
                    wk[:, half * 16 : (half + 1) * 16, :],
                )
            xt0 = load_xt(0)
            wv_sb = singles.tile([128, NKC, HD], bf16)
            nc.gpsimd.dma_start(wv_sb[:], wv[:, :, :])
            wq_sb = xtp.tile([128, NKC, NH_LOC * HD], bf16, bufs=1)
            for half in range(2):
                nc.scalar.dma_start(
                    wq_sb[:, half * 16 : (half + 1) * 16, :],
                    wq[:, half * 16 : (half + 1) * 16, :],
                )
            # all-ones stationary: ones^T @ P^T replicates the per-token key-sum
            # across all 128 PSUM partitions (avoids partition-broadcast later)
            ones_sb = singles.tile([128, 128], bf16)
            nc.vector.memset(ones_sb, 1.0)
            ident_sb = singles.tile([128, 128], bf16)
            make_identity(nc, ident_sb)
            if mode == "causal":
                mask4_sb = singles.tile([128, 4, TQ], bf16)
                nc.scalar.dma_start(mask4_sb[:], mask4[:, :, :])

            # resident K^T [hd, tok], V natural [tk, kb, hd]; Q^T spills to
            # DRAM (tiny contiguous reloads) to keep SBUF headroom
            kres = kvp.tile([128, TOK], bf16)
            vres = kvp.tile([128, TOK // TKB, HD], bf16)
            qres = dram.tile([NH_LOC, NSLOT, 128, TQ], bf16)

            def rope(dst, ps, cos_sl, sin_sl):
                """dst[hd, t] (bf16) <- rotate(ps[hd, t]) with de-interleaved
                halves: rows 0:64 = even lanes (t0), 64:128 = odd lanes (t1).
                out0 = t0*c - t1*s ; out1 = t0*s + t1*c."""
                t0, t1 = ps[0:64], ps[64:128]
                ta = work.tile([64, TQ], f32, tag="rope_a", bufs=2)
                tb = work.tile([64, TQ], f32, tag="rope_b", bufs=2)
                nc.vector.tensor_mul(ta[:], t0, cos_sl[:])
                nc.vector.tensor_mul(tb[:], t1, sin_sl[:])
                nc.vector.tensor_sub(dst[0:64], ta[:], tb[:])
                tc_ = work.tile([64, TQ], f32, tag="rope_a", bufs=2)
                td = work.tile([64, TQ], f32, tag="rope_b", bufs=2)
                nc.vector.tensor_mul(tc_[:], t0, sin_sl[:])
                nc.vector.tensor_mul(td[:], t1, cos_sl[:])
                nc.vector.tensor_add(dst[64:128], tc_[:], td[:])

            # ---- phase A1: all projections, one 512-token slot at a time ----
            for j in range(NSLOT):
                xt_t, cos_sl, sin_sl = xt0 if j == 0 else load_xt(j)

                # K^T
                ps_k = pp.tile([128, TQ], f32, tag="mm")
                for c in range(NKC):
                    nc.tensor.matmul(
                        ps_k[:], wk_sb[:, c, :], xt_t[:, c, :],
                        start=(c == 0), stop=(c == NKC - 1),
                    )
                rope(kres[:, j * TQ : (j + 1) * TQ], ps_k, cos_sl, sin_sl)

                # V^T (transposed to natural after Qproj h=0 fills the PE)
                ps_v = pp.tile([128, TQ], f32, tag="mm")
                for c in range(NKC):
                    nc.tensor.matmul(
                        ps_v[:], wv_sb[:, c, :], xt_t[:, c, :],
                        start=(c == 0), stop=(c == NKC - 1),
                    )
                vt_sb = work.tile([128, TQ], bf16, tag="vt")
                nc.any.tensor_copy(out=vt_sb[:], in_=ps_v[:])

                ps_q = pp.tile([128, TQ], f32, tag="mm")
                for c in range(NKC):
                    nc.tensor.matmul(
                        ps_q[:], wq_sb[:, c, 0:HD], xt_t[:, c, :],
                        start=(c == 0), stop=(c == NKC - 1),
                    )
                qw = work.tile([128, TQ], bf16, tag="qw", bufs=2, name=f"qw{j}_0")
                rope(qw[:], ps_q, cos_sl, sin_sl)
                nc.scalar.dma_start(qres[0, j], qw[:])

                ps_tr = pp.tile([128, TQ], bf16, tag="mm")
                for t in range(TQ // 128):
                    nc.tensor.transpose(
                        ps_tr[:, t * 128 : (t + 1) * 128],
                        vt_sb[:, t * 128 : (t + 1) * 128],
                        ident_sb[:],
                    )
                nc.any.tensor_copy(
                    out=vres[:, j * 4 : j * 4 + 4, :], in_=ps_tr[:]
                )

                for h in range(1, NH_LOC):
                    ps_q = pp.tile([128, TQ], f32, tag="mm", name=f"ps_q{j}_{h}")
                    for c in range(NKC):
                        nc.tensor.matmul(
                            ps_q[:], wq_sb[:, c, h * HD : (h + 1) * HD],
                            xt_t[:, c, :],
                            start=(c == 0), stop=(c == NKC - 1),
                        )
                    qw = work.tile([128, TQ], bf16, tag="qw", bufs=2,
                                   name=f"qw{j}_{h}")
                    rope(qw[:], ps_q, cos_sl, sin_sl)
                    nc.scalar.dma_start(qres[h, j], qw[:])

            # ---- phase A2: attention, head-outer; fire A2A per head ----
            xtctx.close()
            aoctx = ExitStack()
            aop = aoctx.enter_context(tc.tile_pool(name="aop", bufs=1))
            wop = aoctx.enter_context(tc.tile_pool(name="wop", bufs=2))

            def load_wo(nb):
                wo_t = wop.tile([128, NH, TQ], bf16, tag="wot", name=f"wo{nb}")
                for q8 in range(8):  # 8 queues in parallel
                    nc.sync.dma_start(
                        wo_t[:, q8 * 4 : (q8 + 1) * 4, :],
                        wo[nb, :, q8 * 4 : (q8 + 1) * 4, :],
                    )
                return wo_t

            wo_pre = {}
            ao_sb = []
            for h in range(NH_LOC):
                if h == NH_LOC - 1:
                    # prefetch the first two wo blocks under the last head's
                    # attention so phase C starts with weights resident
                    wo_pre[0] = load_wo(0)
                    wo_pre[1] = load_wo(1)
                for b in range(B):
                    for qb in range(NQB):
                        j = b * NQB + qb
                        q_sl = work.tile([128, TQ], bf16, tag="q", bufs=2,
                                         name=f"q{h}_{j}")
                        nc.sync.dma_start(q_sl[:], qres[h, j])
                        q_sl = q_sl[:]
                        nkb = 4 * qb + 4 if mode == "causal" else 4 * NQB
                        kb0 = b * (S // TKB)  # K/V block offset for this batch

                        ps_o = pacc.tile([128, TQ], f32, tag="acc")
                        ps_sum = psums.tile([128, TQ], f32, tag="sums")

                        def scores_pair(pi, j=j, q_sl=q_sl, kb0=kb0, qb=qb):
                            """one exp for two key blocks: [128, 2, TQ]"""
                            ps_s = pp.tile([128, 2, TQ], f32, tag="mm",
                                           name=f"ps_s{j}_{pi}")
                            for i in range(2):
                                kk = kb0 + 2 * pi + i
                                nc.tensor.matmul(
                                    ps_s[:, i, :],
                                    kres[:, kk * TKB : (kk + 1) * TKB],
                                    q_sl, start=True, stop=True,
                                )
                            p_sb = work.tile([128, 2, TQ], bf16, tag="p",
                                             name=f"p_sb{j}_{pi}")
                            if mode == "masked":
                                smask = work.tile([128, 2, TQ], f32, tag="smask")
                                for i in range(2):
                                    kb = 2 * pi + i
                                    nc.sync.dma_start(
                                        smask[:, i, :],
                                        maskT[kb * TKB : (kb + 1) * TKB,
                                              qb * TQ : (qb + 1) * TQ],
                                    )
                                tmp_s = work.tile([128, 2, TQ], f32, tag="tmps")
                                nc.vector.tensor_scalar_mul(tmp_s[:], ps_s[:], SCALE)
                                nc.vector.tensor_add(tmp_s[:], tmp_s[:], smask[:])
                                nc.scalar.activation(
                                    p_sb[:], tmp_s[:],
                                    mybir.ActivationFunctionType.Exp,
                                )
                            else:
                                nc.scalar.activation(
                                    p_sb[:], ps_s[:],
                                    mybir.ActivationFunctionType.Exp,
                                    scale=SCALE,
                                )
                                if mode == "causal" and 2 * pi >= 4 * qb:
                                    nc.vector.tensor_mul(
                                        p_sb[:], p_sb[:],
                                        mask4_sb[:, 2 * (pi - 2 * qb) :
                                                 2 * (pi - 2 * qb) + 2, :],
                                    )
                            return p_sb

                        # prologue: one pair tile, but TWO separate exps
                        # (each ready ~400ns sooner than a joint 1024-wide exp)
                        ps_s01 = pp.tile([128, 2, TQ], f32, tag="mm",
                                         name=f"ps_g{j}")
                        p_sb01 = work.tile([128, 2, TQ], bf16, tag="p",
                                           name=f"p_g{j}")
                        for i in range(2):
                            nc.tensor.matmul(
                                ps_s01[:, i, :],
                                kres[:, (kb0 + i) * TKB : (kb0 + i + 1) * TKB],
                                q_sl, start=True, stop=True,
                            )
                        for i in range(2):
                            if mode == "masked":
                                smask = work.tile([128, 2, TQ], f32, tag="smask",
                                                  name=f"smg{j}_{i}")
                                nc.sync.dma_start(
                                    smask[:, 0, :],
                                    maskT[i * TKB : (i + 1) * TKB,
                                          qb * TQ : (qb + 1) * TQ],
                                )
                                tmp_s = work.tile([128, 2, TQ], f32, tag="tmps",
                                                  name=f"tmg{j}_{i}")
                                nc.vector.tensor_scalar_mul(
                                    tmp_s[:, 0, :], ps_s01[:, i, :], SCALE
                                )
                                nc.vector.tensor_add(
                                    tmp_s[:, 0, :], tmp_s[:, 0, :], smask[:, 0, :]
                                )
                                nc.scalar.activation(
                                    p_sb01[:, i, :], tmp_s[:, 0, :],
                                    mybir.ActivationFunctionType.Exp,
                                )
                            else:
                                nc.scalar.activation(
                                    p_sb01[:, i, :], ps_s01[:, i, :],
                                    mybir.ActivationFunctionType.Exp,
                                    scale=SCALE,
                                )
                                if mode == "causal" and i >= 4 * qb:
                                    nc.vector.tensor_mul(
                                        p_sb01[:, i, :], p_sb01[:, i, :],
                                        mask4_sb[:, i - 4 * qb, :],
                                    )

                        npairs = nkb // 2
                        p_cur = scores_pair(1)
                        for i in range(2):
                            nc.tensor.matmul(
                                ps_o[:], vres[:, kb0 + i, :], p_sb01[:, i, :],
                                start=(i == 0), stop=False,
                            )
                            nc.tensor.matmul(
                                ps_sum[:], ones_sb[:], p_sb01[:, i, :],
                                start=(i == 0), stop=False,
                            )
                        for pi in range(1, npairs):
                            p_next = scores_pair(pi + 1) if pi + 1 < npairs else None
                            for i in range(2):
                                kb = 2 * pi + i
                                kk = kb0 + kb
                                nc.tensor.matmul(
                                    ps_o[:], vres[:, kk, :], p_cur[:, i, :],
                                    start=False, stop=(kb == nkb - 1),
                                )
                                nc.tensor.matmul(
                                    ps_sum[:], ones_sb[:], p_cur[:, i, :],
                                    start=False, stop=(kb == nkb - 1),
                                )
                            p_cur = p_next

                        recip = work.tile([128, TQ], f32, tag="recip", bufs=2)
                        nc.vector.reciprocal_approx_fast(recip[:], ps_sum[:])
                        o_sb = work.tile([128, TQ], bf16, tag="o", bufs=2)
                        nc.vector.tensor_mul(o_sb[:], ps_o[:], recip[:])
                        nc.sync.dma_start(a2a_in[h][j], o_sb[:])

                nc.gpsimd.collective_compute(
                    "AllToAll",
                    mybir.AluOpType.bypass,
                    replica_groups=[list(range(N_CORES))],
                    ins=[a2a_in[h].opt()],
                    outs=[a2a_out[h].opt()],
                )
                ao_t = aop.tile([128, N_CORES, TQ], bf16, name=f"ao_sb{h}")
                for s_ in range(N_CORES):
                    # gpsimd: its queue only carries the (serial) collectives
                    # in A2, so blocking on collective h is harmless here
                    nc.gpsimd.dma_start(ao_t[:, s_, :], a2a_out[h][s_])
                ao_sb.append(ao_t)

            # ---- phase C: out[my 512 tokens] = AO @ wo (full wo) ----
            with (
                tc.tile_pool(name="outp", bufs=3) as outp,
            ):
                for nb in range(D // TQ):
                    wo_t = wo_pre.pop(nb) if nb in wo_pre else load_wo(nb)
                    if nb % 2 == 0:
                        pair_a = pp.tile([128, 2, TQ], f32, tag="mm",
                                         name=f"ps_outa{nb}")
                        pair_b = pp.tile([128, 2, TQ], f32, tag="mm",
                                         name=f"ps_outb{nb}")
                        ps_out = [pair_a[:, 0, :], pair_a[:, 1, :],
                                  pair_b[:, 0, :], pair_b[:, 1, :]]
                    else:
                        # disjoint banks from the even-nb pair tiles: full
                        # overlap of consecutive nb accumulations
                        ps_out = [
                            pacc.tile([128, TQ], f32, tag="acc",
                                      name=f"ps_oa{nb}_{m}")[:]
                            for m in range(2)
                        ] + [
                            psums.tile([128, TQ], f32, tag="sums",
                                       name=f"ps_ob{nb}_{m}")[:]
                            for m in range(2)
                        ]
                    first = True
                    for hg in range(NH_LOC):
                        for s_ in range(N_CORES):
                            k = 4 * s_ + hg
                            last = hg == NH_LOC - 1 and s_ == N_CORES - 1
                            for m in range(4):
                                nc.tensor.matmul(
                                    ps_out[m],
                                    ao_sb[hg][:, s_, m * 128 : (m + 1) * 128],
                                    wo_t[:, k, :],
                                    start=first, stop=last,
                                )
                            first = False
                    for m in range(4):
                        osb = outp.tile([128, TQ], f32, tag="osb")
                        nc.any.tensor_copy(out=osb[:], in_=ps_out[m])
                        nc.scalar.dma_start(
                            out[m * 128 : (m + 1) * 128, nb * TQ : (nb + 1) * TQ],
                            osb[:],
                        )
            aoctx.close()
            psctx.close()
            actx.close()

    nc.finalize()
    return nc


def _detect_mode(mask: np.ndarray) -> str:
    if not np.any(mask):
        return "dense"
    tril_ok = not np.any(mask[np.tril_indices(S)])
    iu = np.triu_indices(S, 1)
    triu_ok = np.all(mask[iu] <= -1e8)
    if tril_ok and triu_ok:
        return "causal"
    return "masked"


def kernel(x, wq, wk, wv, wo, cache_k, cache_v, freqs_cos, freqs_sin, mask,
           start_pos):
    from ml_dtypes import bfloat16

    from concourse.bass_utils import run_bass_kernel_spmd

    assert int(start_pos) == 0, "kernel hardcodes start_pos == 0"
    x = np.asarray(x, dtype=np.float32)
    wq = np.asarray(wq, dtype=np.float32)
    wk = np.asarray(wk, dtype=np.float32)
    wv = np.asarray(wv, dtype=np.float32)
    wo = np.asarray(wo, dtype=np.float32)
    freqs_cos = np.asarray(freqs_cos, dtype=np.float32)
    freqs_sin = np.asarray(freqs_sin, dtype=np.float32)
    mask = np.asarray(mask, dtype=np.float32)

    mode = _detect_mode(mask)
    if mode not in _NC_CACHE:
        _NC_CACHE[mode] = _build_nc(mode)
    nc = _NC_CACHE[mode]

    # X^T slot-tiled [8, 128, 32, 512]: [j, p, c, t] = x_flat[512j+t, 128c+p]
    x_flat = x.reshape(TOK, D)
    xT = np.ascontiguousarray(
        x_flat.T.reshape(NKC, 128, NSLOT, TQ).transpose(2, 1, 0, 3)
    ).astype(bfloat16)

    # de-interleave RoPE pairs within each head: [0,2,...,126,1,3,...,127]
    perm = np.concatenate([np.arange(0, HD, 2), np.arange(1, HD, 2)])

    # cos/sin transposed, tiled over batches: [64, 4096]
    cosT = np.ascontiguousarray(
        np.concatenate([freqs_cos.T] * B, axis=1), dtype=np.float32
    )
    sinT = np.ascontiguousarray(
        np.concatenate([freqs_sin.T] * B, axis=1), dtype=np.float32
    )

    # wo nb-tiled [8, 128, 32, 512]: [nb, p, k, n] = wo[128k+p, 512nb+n]
    wo_bf = np.ascontiguousarray(
        wo.reshape(NH, 128, D // TQ, TQ).transpose(2, 1, 0, 3)
    ).astype(bfloat16)

    def to_chunked(w):  # [4096, F] -> [128, 32, F]
        return np.ascontiguousarray(
            w.reshape(NKC, 128, w.shape[1]).transpose(1, 0, 2)
        ).astype(bfloat16)

    if mode == "causal":
        # mask4[p, c, t] = 1 if t >= 128c + p else 0  (multiplicative, bf16)
        t_idx = np.arange(TQ)[None, None, :]
        p_idx = np.arange(128)[:, None, None]
        c_idx = np.arange(4)[None, :, None]
        mask4 = (t_idx >= 128 * c_idx + p_idx).astype(bfloat16)

    in_maps = []
    for r in range(N_CORES):
        q_cols = np.concatenate(
            [(4 * r + h) * HD + perm for h in range(NH_LOC)]
        )
        m = {
            "xT": xT,
            "wq": to_chunked(wq[:, q_cols]),
            "wk": to_chunked(wk[:, r * HD + perm]),
            "wv": to_chunked(wv[:, r * HD : (r + 1) * HD]),
            "wo": wo_bf,
            "cosT": cosT,
            "sinT": sinT,
        }
        if mode == "causal":
            m["mask4"] = mask4
        if mode == "masked":
            m["maskT"] = np.ascontiguousarray(mask.T)
        in_maps.append(m)

    kwargs = {}
    if PROFILE and TRACE_DIR is not None:
        kwargs["tmpdir"] = TRACE_DIR
    res = run_bass_kernel_spmd(
        nc, in_maps, list(range(N_CORES)), trace=PROFILE, **kwargs
    )
    global LAST_EXEC_NS, LAST_TRACE_DIR
    LAST_EXEC_NS = res.exec_time_ns
    if PROFILE and res.profile_json is not None:
        LAST_TRACE_DIR = res.profile_json

    out_full = np.empty((TOK, D), dtype=np.float32)
    for r in range(N_CORES):
        out_full[r * TQ : (r + 1) * TQ] = res.results[r]["out"]
    return out_full.reshape(B, S, D)


# revision 37
# speedup vs baseline: 1.0642x; 1.0073x over previous
"""Trainium2 8-core tensor-parallel attention kernel (Bass/Tile).

Strategy (TP over heads, per the ColumnParallel/RowParallel intent):
  - Each of the 8 cores owns 1 KV head and its 4 GQA query heads.
  - Phase A1: Q/K/V projections for all 8 512-token slots (bf16 matmuls,
    fp32 PSUM), RoPE on-chip; Q^T/K^T/V kept SBUF-resident.
  - Phase A2: causal attention, head-outer order, no max-subtraction
    (scores are bounded so exp is safe in fp32); O^T = V^T P^T accumulated
    in PSUM over key blocks; per-token softmax denominators via an
    all-ones stationary matmul (replicated across partitions). The key-
    block loop is software-pipelined one deep so the ACT-engine exp never
    stalls the TensorEngine. After each head finishes, a per-head
    AllToAll fires, overlapping later heads' attention.
  - Phase C: the output projection is sequence-parallel: the AllToAlls
    convert head-sharding to token-sharding, then each core multiplies
    its 512-token slice by the FULL wo. No all-reduce; the host
    concatenates the 8 disjoint token slices.

Layout choices:
  - Activations stay transposed (X^T/Q^T/K^T/O^T: features on the 128
    partitions, tokens on the free axis) so every matmul streams 512 wide
    and fp32 DMA-transposes are never needed; V is transposed to natural
    token-major via TensorE transpose (cheap).
  - RoPE pairs are de-interleaved on the host by permuting wq/wk columns
    (even lanes then odd lanes within each head) -> RoPE is 6 vector ops
    on partition halves. The permutation cancels in Q.K^T.
  - All matmul operands are bf16 (fp32 PSUM accumulation).
"""

import math

import numpy as np

B, S, D = 2, 2048, 4096
NH, NKV, HD = 32, 8, 128
N_REP = NH // NKV
N_CORES = 8
TOK = B * S            # 4096 flattened tokens
TQ = 512               # query-block width (matmul moving free dim)
TKB = 128              # key-block width (stationary free dim)
NKC = D // 128         # 32 contraction chunks of 128
NQB = S // TQ          # 4 query blocks per batch
NSLOT = B * NQB        # 8 512-token slots
NH_LOC = NH // N_CORES  # 4 query heads per core
SCALE = 1.0 / math.sqrt(HD)

PROFILE = False
TRACE_DIR = None
LAST_EXEC_NS = None
LAST_TRACE_DIR = None

_NC_CACHE = {}


def _build_nc(mode: str):
    """mode: 'causal' (skip fully-masked blocks, triangular diag masks),
    'dense' (no mask at all), 'masked' (generic additive mask from DRAM)."""
    import concourse.tile as tile
    from concourse import bacc, mybir
    from concourse.masks import make_identity

    f32 = mybir.dt.float32
    bf16 = mybir.dt.bfloat16

    nc = bacc.Bacc(None, target_bir_lowering=False, num_devices=N_CORES)

    xT = nc.declare_dram_parameter("xT", [NSLOT, 128, NKC, TQ], bf16, isOutput=False)
    wq = nc.declare_dram_parameter("wq", [128, NKC, NH_LOC * HD], bf16, isOutput=False)
    wk = nc.declare_dram_parameter("wk", [128, NKC, HD], bf16, isOutput=False)
    wv = nc.declare_dram_parameter("wv", [128, NKC, HD], bf16, isOutput=False)
    wo = nc.declare_dram_parameter("wo", [D // TQ, 128, NH, TQ], bf16, isOutput=False)
    cosT = nc.declare_dram_parameter("cosT", [64, TOK], f32, isOutput=False)
    sinT = nc.declare_dram_parameter("sinT", [64, TOK], f32, isOutput=False)
    if mode == "causal":
        mask4 = nc.declare_dram_parameter("mask4", [128, 4, TQ], bf16, isOutput=False)
    if mode == "masked":
        maskT = nc.declare_dram_parameter("maskT", [S, S], f32, isOutput=False)
    out = nc.declare_dram_parameter("out", [TQ, D], f32, isOutput=True)

    with tile.TileContext(nc) as tc:
        from contextlib import ExitStack

        with (
            tc.tile_pool(name="dram", bufs=1, space="DRAM") as dram,
        ):
            a2a_in = [
                dram.tile([N_CORES, 128, TQ], bf16, name=f"a2a_in{h}")
                for h in range(NH_LOC)
            ]
            a2a_out = [
                dram.tile([N_CORES, 128, TQ], bf16, name=f"a2a_out{h}")
                for h in range(NH_LOC)
            ]

            actx = ExitStack()
            singles = actx.enter_context(tc.tile_pool(name="singles", bufs=1))
            kvp = actx.enter_context(tc.tile_pool(name="kvp", bufs=1))
            work = actx.enter_context(tc.tile_pool(name="work", bufs=3))
            psctx = ExitStack()
            pp = psctx.enter_context(tc.tile_pool(name="pp", bufs=2, space="PSUM"))
            pacc = psctx.enter_context(
                tc.tile_pool(name="pacc", bufs=2, space="PSUM")
            )
            psums = psctx.enter_context(
                tc.tile_pool(name="psums", bufs=2, space="PSUM")
            )
            xtctx = ExitStack()
            xtp = xtctx.enter_context(tc.tile_pool(name="xtp", bufs=2))

            # ---- resident weights/constants, load order = first-use order:
            # wk + slot-0 activations first so the PE starts ~immediately,
            # then wv (needed ~7us in), wq (needed ~14us in), then the rest.
            def load_xt(j):
                # host pre-tiled per slot: per-partition contiguous 32KB
                xt_t = xtp.tile([128, NKC, TQ], bf16, tag="xt", name=f"xt{j}")
                for q4 in range(4):  # 4 queues in parallel
                    nc.sync.dma_start(
                        xt_t[:, q4 * 8 : (q4 + 1) * 8, :],
                        xT[j, :, q4 * 8 : (q4 + 1) * 8, :],
                    )
                cos_sl = work.tile([64, TQ], f32, tag="cos", bufs=2, name=f"cos{j}")
                nc.sync.dma_start(cos_sl[:], cosT[:, j * TQ : (j + 1) * TQ])
                sin_sl = work.tile([64, TQ], f32, tag="sin", bufs=2, name=f"sin{j}")
                nc.sync.dma_start(sin_sl[:], sinT[:, j * TQ : (j + 1) * TQ])
                return xt_t, cos_sl, sin_sl

            wk_sb = singles.tile([128, NKC, HD], bf16)
            for half in range(2):
                nc.sync.dma_start(
                    wk_sb[:, half * 16 : (half + 1) * 16, :],
                    wk[:, half * 16 : (half + 1) * 16, :],
                )
            xt0 = load_xt(0)
            wv_sb = singles.tile([128, NKC, HD], bf16)
            nc.gpsimd.dma_start(wv_sb[:], wv[:, :, :])
            wq_sb = xtp.tile([128, NKC, NH_LOC * HD], bf16, bufs=1)
            for half in range(2):
                nc.scalar.dma_start(
                    wq_sb[:, half * 16 : (half + 1) * 16, :],
                    wq[:, half * 16 : (half + 1) * 16, :],
                )
            # all-ones stationary: ones^T @ P^T replicates the per-token key-sum
            # across all 128 PSUM partitions (avoids partition-broadcast later)
            ones_sb = singles.tile([128, 128], bf16)
            nc.vector.memset(ones_sb, 1.0)
            ident_sb = singles.tile([128, 128], bf16)
            make_identity(nc, ident_sb)
            if mode == "causal":
                mask4_sb = singles.tile([128, 4, TQ], bf16)
                nc.scalar.dma_start(mask4_sb[:], mask4[:, :, :])

            # resident K^T [hd, tok], V natural [tk, kb, hd]; Q^T spills to
            # DRAM (tiny contiguous reloads) to keep SBUF headroom
            kres = kvp.tile([128, TOK], bf16)
            vres = kvp.tile([128, TOK // TKB, HD], bf16)
            qres = dram.tile([NH_LOC, NSLOT, 128, TQ], bf16)

            def rope(dst, ps, cos_sl, sin_sl):
                """dst[hd, t] (bf16) <- rotate(ps[hd, t]) with de-interleaved
                halves: rows 0:64 = even lanes (t0), 64:128 = odd lanes (t1).
                out0 = t0*c - t1*s ; out1 = t0*s + t1*c."""
                t0, t1 = ps[0:64], ps[64:128]
                ta = work.tile([64, TQ], f32, tag="rope_a", bufs=2)
                tb = work.tile([64, TQ], f32, tag="rope_b", bufs=2)
                nc.vector.tensor_mul(ta[:], t0, cos_sl[:])
                nc.vector.tensor_mul(tb[:], t1, sin_sl[:])
                nc.vector.tensor_sub(dst[0:64], ta[:], tb[:])
                tc_ = work.tile([64, TQ], f32, tag="rope_a", bufs=2)
                td = work.tile([64, TQ], f32, tag="rope_b", bufs=2)
                nc.vector.tensor_mul(tc_[:], t0, sin_sl[:])
                nc.vector.tensor_mul(td[:], t1, cos_sl[:])
                nc.vector.tensor_add(dst[64:128], tc_[:], td[:])

            # ---- phase A1: all projections, one 512-token slot at a time ----
            for j in range(NSLOT):
                xt_t, cos_sl, sin_sl = xt0 if j == 0 else load_xt(j)

                # K^T
                ps_k = pp.tile([128, TQ], f32, tag="mm")
                for c in range(NKC):
                    nc.tensor.matmul(
                        ps_k[:], wk_sb[:, c, :], xt_t[:, c, :],
                        start=(c == 0), stop=(c == NKC - 1),
                    )
                rope(kres[:, j * TQ : (j + 1) * TQ], ps_k, cos_sl, sin_sl)

                # V^T (transposed to natural after Qproj h=0 fills the PE)
                ps_v = pp.tile([128, TQ], f32, tag="mm")
                for c in range(NKC):
                    nc.tensor.matmul(
                        ps_v[:], wv_sb[:, c, :], xt_t[:, c, :],
                        start=(c == 0), stop=(c == NKC - 1),
                    )
                vt_sb = work.tile([128, TQ], bf16, tag="vt")
                nc.any.tensor_copy(out=vt_sb[:], in_=ps_v[:])

                ps_q = pp.tile([128, TQ], f32, tag="mm")
                for c in range(NKC):
                    nc.tensor.matmul(
                        ps_q[:], wq_sb[:, c, 0:HD], xt_t[:, c, :],
                        start=(c == 0), stop=(c == NKC - 1),
                    )
                qw = work.tile([128, TQ], bf16, tag="qw", bufs=2, name=f"qw{j}_0")
                rope(qw[:], ps_q, cos_sl, sin_sl)
                nc.scalar.dma_start(qres[0, j], qw[:])

                ps_tr = pp.tile([128, TQ], bf16, tag="mm")
                for t in range(TQ // 128):
                    nc.tensor.transpose(
                        ps_tr[:, t * 128 : (t + 1) * 128],
                        vt_sb[:, t * 128 : (t + 1) * 128],
                        ident_sb[:],
                    )
                nc.any.tensor_copy(
                    out=vres[:, j * 4 : j * 4 + 4, :], in_=ps_tr[:]
                )

                for h in range(1, NH_LOC):
                    ps_q = pp.tile([128, TQ], f32, tag="mm", name=f"ps_q{j}_{h}")
                    for c in range(NKC):
                        nc.tensor.matmul(
                            ps_q[:], wq_sb[:, c, h * HD : (h + 1) * HD],
                            xt_t[:, c, :],
                            start=(c == 0), stop=(c == NKC - 1),
                        )
                    qw = work.tile([128, TQ], bf16, tag="qw", bufs=2,
                                   name=f"qw{j}_{h}")
                    rope(qw[:], ps_q, cos_sl, sin_sl)
                    nc.scalar.dma_start(qres[h, j], qw[:])

            # ---- phase A2: attention, head-outer; fire A2A per head ----
            xtctx.close()
            aoctx = ExitStack()
            aop = aoctx.enter_context(tc.tile_pool(name="aop", bufs=1))
            wop = aoctx.enter_context(tc.tile_pool(name="wop", bufs=2))

            def load_wo(nb):
                wo_t = wop.tile([128, NH, TQ], bf16, tag="wot", name=f"wo{nb}")
                for q8 in range(8):  # 8 queues in parallel
                    nc.sync.dma_start(
                        wo_t[:, q8 * 4 : (q8 + 1) * 4, :],
                        wo[nb, :, q8 * 4 : (q8 + 1) * 4, :],
                    )
                return wo_t

            wo_pre = {}
            ao_sb = []
            for h in range(NH_LOC):
                if h == NH_LOC - 1:
                    # prefetch the first two wo blocks under the last head's
                    # attention so phase C starts with weights resident
                    wo_pre[0] = load_wo(0)
                    wo_pre[1] = load_wo(1)
                for b in range(B):
                    for qb in range(NQB):
                        j = b * NQB + qb
                        q_sl = work.tile([128, TQ], bf16, tag="q", bufs=2,
                                         name=f"q{h}_{j}")
                        nc.sync.dma_start(q_sl[:], qres[h, j])
                        q_sl = q_sl[:]
                        nkb = 4 * qb + 4 if mode == "causal" else 4 * NQB
                        kb0 = b * (S // TKB)  # K/V block offset for this batch

                        ps_o = pacc.tile([128, TQ], f32, tag="acc")
                        ps_sum = psums.tile([128, TQ], f32, tag="sums")

                        def scores_pair(pi, j=j, q_sl=q_sl, kb0=kb0, qb=qb):
                            """one exp for two key blocks: [128, 2, TQ]"""
                            ps_s = pp.tile([128, 2, TQ], f32, tag="mm",
                                           name=f"ps_s{j}_{pi}")
                            for i in range(2):
                                kk = kb0 + 2 * pi + i
                                nc.tensor.matmul(
                                    ps_s[:, i, :],
                                    kres[:, kk * TKB : (kk + 1) * TKB],
                                    q_sl, start=True, stop=True,
                                )
                            p_sb = work.tile([128, 2, TQ], bf16, tag="p",
                                             name=f"p_sb{j}_{pi}")
                            if mode == "masked":
                                smask = work.tile([128, 2, TQ], f32, tag="smask")
                                for i in range(2):
                                    kb = 2 * pi + i
                                    nc.sync.dma_start(
                                        smask[:, i, :],
                                        maskT[kb * TKB : (kb + 1) * TKB,
                                              qb * TQ : (qb + 1) * TQ],
                                    )
                                tmp_s = work.tile([128, 2, TQ], f32, tag="tmps")
                                nc.vector.tensor_scalar_mul(tmp_s[:], ps_s[:], SCALE)
                                nc.vector.tensor_add(tmp_s[:], tmp_s[:], smask[:])
                                nc.scalar.activation(
                                    p_sb[:], tmp_s[:],
                                    mybir.ActivationFunctionType.Exp,
                                )
                            else:
                                nc.scalar.activation(
                                    p_sb[:], ps_s[:],
                                    mybir.ActivationFunctionType.Exp,
                                    scale=SCALE,
                                )
                                if mode == "causal" and 2 * pi >= 4 * qb:
                                    nc.vector.tensor_mul(
                                        p_sb[:], p_sb[:],
                                        mask4_sb[:, 2 * (pi - 2 * qb) :
                                                 2 * (pi - 2 * qb) + 2, :],
                                    )
                            return p_sb

                        # prologue: one pair tile, but TWO separate exps
                        # (each ready ~400ns sooner than a joint 1024-wide exp)
                        ps_s01 = pp.tile([128, 2, TQ], f32, tag="mm",
                                         name=f"ps_g{j}")
                        p_sb01 = work.tile([128, 2, TQ], bf16, tag="p",
                                           name=f"p_g{j}")
                        for i in range(2):
                            nc.tensor.matmul(
                                ps_s01[:, i, :],
                                kres[:, (kb0 + i) * TKB : (kb0 + i + 1) * TKB],
                                q_sl, start=True, stop=True,
                            )
                        for i in range(2):
                            if mode == "masked":
                                smask = work.tile([128, 2, TQ], f32, tag="smask",
                                                  name=f"smg{j}_{i}")
                                nc.sync.dma_start(
                                    smask[:, 0, :],
                                    maskT[i * TKB : (i + 1) * TKB,
                                          qb * TQ : (qb + 1) * TQ],
                                )
                                tmp_s = work.tile([128, 2, TQ], f32, tag="tmps",
                                                  name=f"tmg{j}_{i}")
                                nc.vector.tensor_scalar_mul(
                                    tmp_s[:, 0, :], ps_s01[:, i, :], SCALE
                                )
                                nc.vector.tensor_add(
                                    tmp_s[:, 0, :], tmp_s[:, 0, :], smask[:, 0, :]
                                )
                                nc.scalar.activation(
                                    p_sb01[:, i, :], tmp_s[:, 0, :],
                                    mybir.ActivationFunctionType.Exp,
                                )
                            else:
                                nc.scalar.activation(
                                    p_sb01[:, i, :], ps_s01[:, i, :],
                                    mybir.ActivationFunctionType.Exp,
                                    scale=SCALE,
                                )
                                if mode == "causal" and i >= 4 * qb:
                                    nc.vector.tensor_mul(
                                        p_sb01[:, i, :], p_sb01[:, i, :],
                                        mask4_sb[:, i - 4 * qb, :],
                                    )

                        npairs = nkb // 2
                        p_cur = scores_pair(1)
                        for i in range(2):
                            nc.tensor.matmul(
                                ps_o[:], vres[:, kb0 + i, :], p_sb01[:, i, :],
                                start=(i == 0), stop=False,
                            )
                            nc.tensor.matmul(
                                ps_sum[:], ones_sb[:], p_sb01[:, i, :],
                                start=(i == 0), stop=False,
                            )
                        for pi in range(1, npairs):
                            p_next = scores_pair(pi + 1) if pi + 1 < npairs else None
                            for i in range(2):
                                kb = 2 * pi + i
                                kk = kb0 + kb
                                nc.tensor.matmul(
                                    ps_o[:], vres[:, kk, :], p_cur[:, i, :],
                                    start=False, stop=(kb == nkb - 1),
                                )
                                nc.tensor.matmul(
                                    ps_sum[:], ones_sb[:], p_cur[:, i, :],
                                    start=False, stop=(kb == nkb - 1),
                                )
                            p_cur = p_next

                        recip = work.tile([128, TQ], f32, tag="recip", bufs=2)
                        nc.vector.reciprocal_approx_fast(recip[:], ps_sum[:])
                        o_sb = work.tile([128, TQ], bf16, tag="o", bufs=2)
                        nc.vector.tensor_mul(o_sb[:], ps_o[:], recip[:])
                        nc.sync.dma_start(a2a_in[h][j], o_sb[:])

                nc.gpsimd.collective_compute(
                    "AllToAll",
                    mybir.AluOpType.bypass,
                    replica_groups=[list(range(N_CORES))],
                    ins=[a2a_in[h].opt()],
                    outs=[a2a_out[h].opt()],
                )
                ao_t = aop.tile([128, N_CORES, TQ], bf16, name=f"ao_sb{h}")
                for s_ in range(N_CORES):
                    # gpsimd: its queue only carries the (serial) collectives
                    # in A2, so blocking on collective h is harmless here
                    nc.gpsimd.dma_start(ao_t[:, s_, :], a2a_out[h][s_])
                ao_sb.append(ao_t)

            # ---- phase C: out[my 512 tokens] = AO @ wo (full wo) ----
            with (
                tc.tile_pool(name="outp", bufs=3) as outp,
            ):
                def alloc_ps_out(nb):
                    if nb % 2 == 0:
                        pair_a = pp.tile([128, 2, TQ], f32, tag="mm",
                                         name=f"ps_outa{nb}")
                        pair_b = pp.tile([128, 2, TQ], f32, tag="mm",
                                         name=f"ps_outb{nb}")
                        return [pair_a[:, 0, :], pair_a[:, 1, :],
                                pair_b[:, 0, :], pair_b[:, 1, :]]
                    return [
                        pacc.tile([128, TQ], f32, tag="acc",
                                  name=f"ps_oa{nb}_{m}")[:]
                        for m in range(2)
                    ] + [
                        psums.tile([128, TQ], f32, tag="sums",
                                   name=f"ps_ob{nb}_{m}")[:]
                        for m in range(2)
                    ]

                def mm_hgs(ps_out, wo_t, hgs, first):
                    for hg in hgs:
                        for s_ in range(N_CORES):
                            k = 4 * s_ + hg
                            last = hg == NH_LOC - 1 and s_ == N_CORES - 1
                            for m in range(4):
                                nc.tensor.matmul(
                                    ps_out[m],
                                    ao_sb[hg][:, s_, m * 128 : (m + 1) * 128],
                                    wo_t[:, k, :],
                                    start=first, stop=last,
                                )
                            first = False

                def flush(nb, ps_out):
                    for m in range(4):
                        osb = outp.tile([128, TQ], f32, tag="osb",
                                        name=f"osb{nb}_{m}")
                        nc.any.tensor_copy(out=osb[:], in_=ps_out[m])
                        nc.scalar.dma_start(
                            out[m * 128 : (m + 1) * 128, nb * TQ : (nb + 1) * TQ],
                            osb[:],
                        )

                # nb 0 and 1 run head-groups 0-2 first (disjoint PSUM banks),
                # deferring their hg3 so the last AllToAll has ~50us to land
                wo_t0 = wo_pre.pop(0)
                ps0 = alloc_ps_out(0)
                mm_hgs(ps0, wo_t0, range(NH_LOC - 1), True)
                wo_t1 = wo_pre.pop(1)
                ps1 = alloc_ps_out(1)
                mm_hgs(ps1, wo_t1, range(NH_LOC - 1), True)
                mm_hgs(ps0, wo_t0, [NH_LOC - 1], False)
                flush(0, ps0)
                mm_hgs(ps1, wo_t1, [NH_LOC - 1], False)
                flush(1, ps1)
                for nb in range(2, D // TQ):
                    wo_t = load_wo(nb)
                    ps_out = alloc_ps_out(nb)
                    mm_hgs(ps_out, wo_t, range(NH_LOC), True)
                    flush(nb, ps_out)
            aoctx.close()
            psctx.close()
            actx.close()

    nc.finalize()
    return nc


def _detect_mode(mask: np.ndarray) -> str:
    if not np.any(mask):
        return "dense"
    tril_ok = not np.any(mask[np.tril_indices(S)])
    iu = np.triu_indices(S, 1)
    triu_ok = np.all(mask[iu] <= -1e8)
    if tril_ok and triu_ok:
        return "causal"
    return "masked"


def kernel(x, wq, wk, wv, wo, cache_k, cache_v, freqs_cos, freqs_sin, mask,
           start_pos):
    from ml_dtypes import bfloat16

    from concourse.bass_utils import run_bass_kernel_spmd

    assert int(start_pos) == 0, "kernel hardcodes start_pos == 0"
    x = np.asarray(x, dtype=np.float32)
    wq = np.asarray(wq, dtype=np.float32)
    wk = np.asarray(wk, dtype=np.float32)
    wv = np.asarray(wv, dtype=np.float32)
    wo = np.asarray(wo, dtype=np.float32)
    freqs_cos = np.asarray(freqs_cos, dtype=np.float32)
    freqs_sin = np.asarray(freqs_sin, dtype=np.float32)
    mask = np.asarray(mask, dtype=np.float32)

    mode = _detect_mode(mask)
    if mode not in _NC_CACHE:
        _NC_CACHE[mode] = _build_nc(mode)
    nc = _NC_CACHE[mode]

    # X^T slot-tiled [8, 128, 32, 512]: [j, p, c, t] = x_flat[512j+t, 128c+p]
    x_flat = x.reshape(TOK, D)
    xT = np.ascontiguousarray(
        x_flat.T.reshape(NKC, 128, NSLOT, TQ).transpose(2, 1, 0, 3)
    ).astype(bfloat16)

    # de-interleave RoPE pairs within each head: [0,2,...,126,1,3,...,127]
    perm = np.concatenate([np.arange(0, HD, 2), np.arange(1, HD, 2)])

    # cos/sin transposed, tiled over batches: [64, 4096]
    cosT = np.ascontiguousarray(
        np.concatenate([freqs_cos.T] * B, axis=1), dtype=np.float32
    )
    sinT = np.ascontiguousarray(
        np.concatenate([freqs_sin.T] * B, axis=1), dtype=np.float32
    )

    # wo nb-tiled [8, 128, 32, 512]: [nb, p, k, n] = wo[128k+p, 512nb+n]
    wo_bf = np.ascontiguousarray(
        wo.reshape(NH, 128, D // TQ, TQ).transpose(2, 1, 0, 3)
    ).astype(bfloat16)

    def to_chunked(w):  # [4096, F] -> [128, 32, F]
        return np.ascontiguousarray(
            w.reshape(NKC, 128, w.shape[1]).transpose(1, 0, 2)
        ).astype(bfloat16)

    if mode == "causal":
        # mask4[p, c, t] = 1 if t >= 128c + p else 0  (multiplicative, bf16)
        t_idx = np.arange(TQ)[None, None, :]
        p_idx = np.arange(128)[:, None, None]
        c_idx = np.arange(4)[None, :, None]
        mask4 = (t_idx >= 128 * c_idx + p_idx).astype(bfloat16)

    in_maps = []
    for r in range(N_CORES):
        q_cols = np.concatenate(
            [(4 * r + h) * HD + perm for h in range(NH_LOC)]
        )
        m = {
            "xT": xT,
            "wq": to_chunked(wq[:, q_cols]),
            "wk": to_chunked(wk[:, r * HD + perm]),
            "wv": to_chunked(wv[:, r * HD : (r + 1) * HD]),
            "wo": wo_bf,
            "cosT": cosT,
            "sinT": sinT,
        }
        if mode == "causal":
            m["mask4"] = mask4
        if mode == "masked":
            m["maskT"] = np.ascontiguousarray(mask.T)
        in_maps.append(m)

    kwargs = {}
    if PROFILE and TRACE_DIR is not None:
        kwargs["tmpdir"] = TRACE_DIR
    res = run_bass_kernel_spmd(
        nc, in_maps, list(range(N_CORES)), trace=PROFILE, **kwargs
    )
    global LAST_EXEC_NS, LAST_TRACE_DIR
    LAST_EXEC_NS = res.exec_time_ns
    if PROFILE and res.profile_json is not None:
        LAST_TRACE_DIR = res.profile_json

    out_full = np.empty((TOK, D), dtype=np.float32)
    for r in range(N_CORES):
        out_full[r * TQ : (r + 1) * TQ] = res.results[r]["out"]
    return out_full.reshape(B, S, D)
